# revision 37
# baseline (speedup 1.0000x reference)
"""Multi-head causal attention (RoPE) on 8 Trainium2 NeuronCores.

Sharding (Megatron-style): core c handles batch c//4 and the 4 heads
[4*(c%4), 4*(c%4)+4). Each core computes Q/K/V projections for its
head slice, rotary embedding, causal flash-style attention (no
max-subtraction: scores are O(10) so exp is safe), and its partial
output projection through the matching Wo column block. The host sums
the 4 partial outputs per batch and transposes (the device computes
out.T: [model_dim, seq], bf16).

All on-device layouts are transposed ([feature, seq]) so that
- projections use hsT tiles as the moving operand (N=512 matmuls),
- QK^T produces scores.T directly ([key, query]) which is what the
  AV matmul wants as its moving operand, and
- softmax normalization uses a ones-matmul partition-sum that also
  broadcasts the denominator across partitions.

Optimizations vs the 382us baseline (measured 327us):
- rope uses a sign-folded [-sin; sin] table: 3 muls + 1 add per
  projection group instead of 6 ops (DVE was the projection-chain
  serializer at 3.8us/group > the 3.4us matmul group).
- causal diagonal tiles narrow the query range to q >= (j-4ic)*128:
  fewer QK/AV matmul columns, fewer exp'd elements, and the band
  mask shrinks to a [128,128] tril on the leading columns.
- two-pass schedule: K+V projections for all strips first, then Q
  projections in REVERSE chunk order, each followed by its chunk's
  attention + Wo block. The heaviest causal chunk's long EXP chain
  starts first and hides under the remaining Q/Wo matmuls; the
  kernel ends on the lightest chunk (no ACT-bound tail hump).
  Strips for the two largest chunks stay resident; the other two
  re-load (+4.2MB DMA, fully hidden).
- warmup sized (48 matmuls) to keep the PE busy and the HAM clock
  gate at 8/8 exactly until the gating DMAs (strip0 + Wk) land.
- PSUM: pp=2, sp=3 (score tiles; lets QK run 2 ahead of EXP),
  av=1, misc=2 banks.
- output is bf16 (halves the store DMA; host sums partials in f32).
"""

import os

import numpy as np
import ml_dtypes

import concourse.bass as bass
import concourse.mybir as mybir
import concourse.tile as tile
from concourse import bacc
from concourse.bass_utils import run_bass_kernel_spmd

BF16 = mybir.dt.bfloat16
F16 = mybir.dt.float16
F32 = mybir.dt.float32
NPBF16 = ml_dtypes.bfloat16
NPF16 = np.float16

NCORES = 8
B = 2
S = 2048
HDIM = 2048
NH = 16
HD = 128
HPC = 4  # heads per core
CPB = 4  # cores per batch
SCW = 512  # s-chunk width
NSC = S // SCW  # 4
KT = HDIM // 128  # 16 k-tiles
NJT = S // 128  # 16 j-tiles
SCALE = 1.0 / np.sqrt(HD)
ROPE_BASE = 10000.0
WARM_MMS = 54

_NC_CACHE: dict[str, object] = {}
LAST_EXEC_TIME_NS = None


def _build(mode: str):
    """mode: 'causal' | 'full' | 'general'"""
    nc = bacc.Bacc("TRN2", target_bir_lowering=False, debug=False,
                   num_devices=NCORES)

    hst_d = nc.declare_dram_parameter("hst", [NSC, 128, KT * SCW], BF16, isOutput=False)
    wq_d = nc.declare_dram_parameter("wq", [128, HPC * KT * 128], BF16, isOutput=False)
    wk_d = nc.declare_dram_parameter("wk", [128, HPC * KT * 128], BF16, isOutput=False)
    wv_d = nc.declare_dram_parameter("wv", [128, KT * 512], BF16, isOutput=False)
    wo_d = nc.declare_dram_parameter("wo", [128, HPC * KT * 128], BF16, isOutput=False)
    cos_d = nc.declare_dram_parameter("cosd", [128, S], BF16, isOutput=False)
    sin_d = nc.declare_dram_parameter("sind", [128, S], BF16, isOutput=False)
    tril_d = nc.declare_dram_parameter("tril", [128, 128], F16, isOutput=False)
    if mode == "general":
        em_d = nc.declare_dram_parameter("emask", [S, S], F16, isOutput=False)
    out_d = nc.declare_dram_parameter("outT", [HDIM, S], BF16, isOutput=True)

    Exp = mybir.ActivationFunctionType.Exp

    with tile.TileContext(nc) as tc:
        with (
            tc.tile_pool(name="wpool", bufs=1) as wpool,
            tc.tile_pool(name="cpool", bufs=1) as cpool,
            tc.tile_pool(name="qkv", bufs=1) as qkvp,
            tc.tile_pool(name="strip", bufs=2) as stripp,
            tc.tile_pool(name="ropet", bufs=2) as ropet,
            tc.tile_pool(name="probs", bufs=4) as probsp,
            tc.tile_pool(name="psums", bufs=2) as psums,
            tc.tile_pool(name="recips", bufs=2) as recips,
            tc.tile_pool(name="fouts", bufs=3) as fouts,
            tc.tile_pool(name="emt", bufs=4) as emtp,
            tc.tile_pool(name="pp", bufs=2, space="PSUM") as pp_pool,
            tc.tile_pool(name="sp", bufs=3, space="PSUM") as sp_pool,
            tc.tile_pool(name="av", bufs=1, space="PSUM") as av_pool,
            tc.tile_pool(name="misc_ps", bufs=2, space="PSUM") as misc_ps,
        ):
            wq = wpool.tile([128, HPC * KT * 128], BF16, tag="wq")
            wk = wpool.tile([128, HPC * KT * 128], BF16, tag="wk")
            wv = wpool.tile([128, KT * 512], BF16, tag="wv")
            wo = wpool.tile([128, HPC * KT * 128], BF16, tag="wo")
            cos = cpool.tile([128, S], BF16, tag="cos")   # [cos; cos]
            sin = cpool.tile([128, S], BF16, tag="sin")   # [-sin; sin]
            tril = cpool.tile([128, 128], F16, tag="tril")
            ones = cpool.tile([128, 128], F16, tag="ones")

            qT = qkvp.tile([128, HPC * S], BF16, tag="qT")
            kTt = qkvp.tile([128, HPC * S], BF16, tag="kT")
            vN = qkvp.tile([128, NJT * 512], F16, tag="vN")
            oT = qkvp.tile([128, HPC * NSC * 512], BF16, tag="oT")

            # First strip + pass-1 weights are the startup-critical DMAs;
            # issue them BEFORE the gpsimd memsets so their SWDGE
            # descriptor generation (also on the Q7 cores) isn't queued
            # behind the memsets.
            strip0 = stripp.tile([128, KT * SCW], BF16, tag="st", name="strip")
            nc.sync.dma_start(strip0[:], hst_d[0])
            nc.sync.dma_start(wk[:], wk_d[:])
            nc.sync.dma_start(cos[:], cos_d[:])
            nc.sync.dma_start(sin[:], sin_d[:])
            nc.sync.dma_start(wv[:], wv_d[:])
            nc.sync.dma_start(tril[:], tril_d[:])

            nc.gpsimd.memset(ones[:], 1.0)
            # PE warmup: keep the PE busy ~3.5us+ so the HAM clock-gate
            # reaches 8/8 while the first strip + Wk DMAs land.
            warm = cpool.tile([128, 512], BF16, tag="warm")
            nc.gpsimd.memset(warm[:, 0:128], 0.0)
            wps = misc_ps.tile([128, SCW], F32, tag="mp")
            for _ in range(WARM_MMS):
                nc.tensor.matmul(wps[:], warm[:, 0:128], warm[:],
                                 start=True, stop=True)

            def proj_group(wt, dst, strip, sc, h):
                """One 16-matmul projection group + sign-folded rope.

                rope: A = pq * [cos;cos]; Bt[0:64] = pq[64:128] * (-sin);
                Bt[64:128] = pq[0:64] * (+sin); dst = A + Bt.
                (walrus only requires equal base partitions when BOTH
                inputs are SBUF, so the half-shifted reads go through the
                PSUM operand.)
                """
                pq = pp_pool.tile([128, SCW], F32, tag="pp", name="pq")
                for kt in range(KT):
                    nc.tensor.matmul(
                        pq[:],
                        wt[:, (h * KT + kt) * 128:(h * KT + kt + 1) * 128],
                        strip[:, kt * SCW:(kt + 1) * SCW],
                        start=(kt == 0), stop=(kt == KT - 1),
                    )
                cs = cos[:, sc * SCW:(sc + 1) * SCW]
                sn = sin[:, sc * SCW:(sc + 1) * SCW]
                dd = dst[:, h * S + sc * SCW: h * S + (sc + 1) * SCW]
                t1 = ropet.tile([128, SCW], BF16, tag="t1", name="t1")
                t2 = ropet.tile([128, SCW], BF16, tag="t2", name="t2")
                nc.vector.tensor_mul(t1[:], pq[:], cs)
                nc.vector.tensor_mul(t2[0:64, :], pq[64:128, :], sn[0:64, :])
                nc.vector.tensor_mul(t2[64:128, :], pq[0:64, :], sn[64:128, :])
                nc.vector.tensor_add(dd, t1[:], t2[:])

            # Pass 1: K + V projections for all strips (no attention yet).
            kept = {}
            for sc in range(NSC):
                if sc == 0:
                    strip = strip0
                else:
                    strip = stripp.tile([128, KT * SCW], BF16, tag="st",
                                        name="strip")
                    nc.sync.dma_start(strip[:], hst_d[sc])
                kept[sc] = strip
                for h in range(HPC):
                    proj_group(wk, kTt, strip, sc, h)
                for st in range(4):
                    vp = pp_pool.tile([128, SCW], F32, tag="pp", name="vp")
                    for kt in range(KT):
                        nc.tensor.matmul(
                            vp[:],
                            strip[:, kt * SCW + st * 128: kt * SCW + (st + 1) * 128],
                            wv[:, kt * 512:(kt + 1) * 512],
                            start=(kt == 0), stop=(kt == KT - 1),
                        )
                    jt = sc * 4 + st
                    nc.scalar.copy(vN[:, jt * 512:(jt + 1) * 512], vp[:])

            def emit_wo(ic):
                for mt in range(KT):
                    fp = misc_ps.tile([128, SCW], F32, tag="mp", name="fp")
                    for h in range(HPC):
                        nc.tensor.matmul(
                            fp[:],
                            wo[:, (h * KT + mt) * 128:(h * KT + mt + 1) * 128],
                            oT[:, (h * NSC + ic) * 512:(h * NSC + ic + 1) * 512],
                            start=(h == 0), stop=(h == HPC - 1),
                        )
                    fs = fouts.tile([128, SCW], BF16, tag="fs", name="fs")
                    nc.scalar.copy(fs[:], fp[:])
                    nc.sync.dma_start(
                        out_d[mt * 128:(mt + 1) * 128, ic * SCW:(ic + 1) * SCW],
                        fs[:],
                    )

            # Pass 2: Q projection + attention + Wo, in REVERSE chunk order.
            # The heaviest causal chunk (ic = NSC-1) starts first, so its
            # long EXP chain overlaps the remaining Q projections and Wo
            # blocks; the kernel ends on the lightest chunk. Strips for the
            # two largest chunks are still resident from pass 1; the other
            # two are re-loaded (the strip pool recycles their slots).
            for ic in reversed(range(NSC)):
                if ic == NSC - 1:
                    nc.sync.dma_start(wq[:], wq_d[:])
                    nc.sync.dma_start(wo[:], wo_d[:])
                if ic >= NSC - 2:
                    strip = kept[ic]
                else:
                    strip = stripp.tile([128, KT * SCW], BF16, tag="st",
                                        name="strip")
                    nc.sync.dma_start(strip[:], hst_d[ic])
                for h in range(HPC):
                    proj_group(wq, qT, strip, ic, h)

                nj = 4 * (ic + 1) if mode == "causal" else NJT
                for h in range(HPC):
                    av = av_pool.tile([128, SCW], F32)
                    Ps = psums.tile([128, SCW], F16)
                    for j in range(nj):
                        q0 = ((j - 4 * ic) * 128
                              if mode == "causal" and j >= 4 * ic else 0)
                        sp = sp_pool.tile([128, SCW], F32)
                        nc.tensor.matmul(
                            sp[:, q0:512],
                            kTt[:, h * S + j * 128: h * S + (j + 1) * 128],
                            qT[:, h * S + ic * SCW + q0: h * S + (ic + 1) * SCW],
                            start=True, stop=True,
                        )
                        pr = probsp.tile([128, SCW], F16)
                        nc.scalar.activation(pr[:, q0:512], sp[:, q0:512],
                                             Exp, scale=float(SCALE))
                        if mode == "general":
                            emt = emtp.tile([128, SCW], F16)
                            nc.sync.dma_start(
                                emt[:],
                                em_d[j * 128:(j + 1) * 128,
                                     ic * SCW:(ic + 1) * SCW],
                            )
                            nc.vector.tensor_mul(pr[:], pr[:], emt[:])
                        elif mode == "causal" and j >= 4 * ic:
                            # within-tile causal triangle on the leading
                            # 128 query columns of the narrowed range
                            nc.vector.tensor_mul(pr[:, q0:q0 + 128],
                                                 pr[:, q0:q0 + 128],
                                                 tril[:])
                        if j == 0:
                            nc.vector.tensor_copy(Ps[:], pr[:])
                        else:
                            nc.vector.tensor_add(Ps[:, q0:512],
                                                 Ps[:, q0:512],
                                                 pr[:, q0:512])
                        nc.tensor.matmul(
                            av[:, q0:512],
                            vN[:, j * 512 + h * 128: j * 512 + (h + 1) * 128],
                            pr[:, q0:512],
                            start=(j == 0), stop=(j == nj - 1),
                        )
                    rs = misc_ps.tile([128, SCW], F32, tag="mp")
                    nc.tensor.matmul(rs[:], ones[:], Ps[:], start=True, stop=True)
                    rc = recips.tile([128, SCW], F32)
                    nc.vector.reciprocal_approx_fast(rc[:], rs[:])
                    nc.vector.tensor_mul(
                        oT[:, (h * NSC + ic) * 512:(h * NSC + ic + 1) * 512],
                        av[:], rc[:],
                    )
                emit_wo(ic)

    nc.compile()
    return nc


def _get_nc(mode: str):
    if mode not in _NC_CACHE:
        _NC_CACHE[mode] = _build(mode)
    return _NC_CACHE[mode]


def _classify_mask(m: np.ndarray) -> str:
    if not m.any():
        return "full"
    tril = np.tril(np.ones((S, S), dtype=bool))
    if np.all(m[tril] == 0.0) and np.all(m[~tril] <= -1e8):
        return "causal"
    return "general"


def kernel(hidden_states, attention_mask, position_ids, Wq, Wk, Wv, Wo):
    global LAST_EXEC_TIME_NS
    hs = np.asarray(hidden_states, dtype=np.float32)
    mask = np.asarray(attention_mask, dtype=np.float32)[0, 0]
    pos = np.asarray(position_ids)
    Wq = np.asarray(Wq, dtype=np.float32)
    Wk = np.asarray(Wk, dtype=np.float32)
    Wv = np.asarray(Wv, dtype=np.float32)
    Wo = np.asarray(Wo, dtype=np.float32)

    mode = _classify_mask(mask)
    nc = _get_nc(mode)

    # rope tables per batch: emb = concat(freqs, freqs) -> the two head-dim
    # halves share the same angle table. Sign-folded for the device:
    #   cosd = [cos; cos],  sind = [-sin; sin]   (both [128, S])
    inv_freq = 1.0 / (ROPE_BASE ** (np.arange(0, HD, 2, dtype=np.float32) / HD))
    cos_b, sin_b = [], []
    for b in range(B):
        ang = np.outer(pos[b].astype(np.float32), inv_freq)  # [S, 64]
        c = np.cos(ang).T.astype(np.float32)                 # [64, S]
        s = np.sin(ang).T.astype(np.float32)
        cos_b.append(np.concatenate([c, c], axis=0).astype(NPBF16).copy())
        sin_b.append(np.concatenate([-s, s], axis=0).astype(NPBF16).copy())

    # within-tile causal triangle: tril[p, x] = 1 if p <= x
    pidx = np.arange(128)[:, None]
    xidx = np.arange(128)[None, :]
    trilm = (pidx <= xidx).astype(NPF16)

    emask = None
    if mode == "general":
        with np.errstate(under="ignore", over="ignore"):
            emask = np.exp(mask.T.astype(np.float64)).astype(NPF16)

    in_maps = []
    for c in range(NCORES):
        b = c // CPB
        r0 = (c % CPB) * HPC * HD  # feature-row base of this core's heads

        hsb = hs[b]  # [S, HDIM]
        hst = (hsb.reshape(NSC, SCW, KT, 128).transpose(0, 3, 2, 1)
               .reshape(NSC, 128, KT * SCW).astype(NPBF16))

        Wq_s = Wq[r0:r0 + 512]  # [512, HDIM]
        wq_t = (Wq_s.reshape(HPC, 128, KT, 128).transpose(3, 0, 2, 1)
                .reshape(128, HPC * KT * 128).astype(NPBF16))
        Wk_s = Wk[r0:r0 + 512]
        wk_t = (Wk_s.reshape(HPC, 128, KT, 128).transpose(3, 0, 2, 1)
                .reshape(128, HPC * KT * 128).astype(NPBF16))
        Wv_s = Wv[r0:r0 + 512]  # [512, HDIM]
        wv_t = (Wv_s.reshape(512, KT, 128).transpose(2, 1, 0)
                .reshape(128, KT * 512).astype(NPBF16))
        Wo_s = Wo[:, r0:r0 + 512]  # [HDIM, 512]
        wo_t = (Wo_s.reshape(KT, 128, HPC, 128).transpose(3, 2, 0, 1)
                .reshape(128, HPC * KT * 128).astype(NPBF16))

        m = {
            "hst": hst, "wq": wq_t, "wk": wk_t, "wv": wv_t, "wo": wo_t,
            "cosd": cos_b[b], "sind": sin_b[b], "tril": trilm,
        }
        if mode == "general":
            m["emask"] = emask
        in_maps.append(m)

    trace = os.environ.get("BASS_KERNEL_TRACE") == "1"
    res = run_bass_kernel_spmd(nc, in_maps, core_ids=list(range(NCORES)),
                               trace=trace)
    LAST_EXEC_TIME_NS = res.exec_time_ns

    out = np.empty((B, S, HDIM), dtype=np.float32)
    for b in range(B):
        acc = res.results[CPB * b]["outT"].astype(np.float32)
        for c in range(CPB * b + 1, CPB * (b + 1)):
            acc = acc + res.results[c]["outT"].astype(np.float32)
        out[b] = acc.T
    return out


# revision 39
# speedup vs baseline: 1.1880x; 1.1880x over previous
"""Multi-head causal attention (RoPE) on 8 Trainium2 NeuronCores.

Sharding (Megatron-style): core c handles batch c//4 and the 4 heads
[4*(c%4), 4*(c%4)+4). Each core computes Q/K/V projections for its
head slice, rotary embedding, causal flash-style attention (no
max-subtraction: scores are O(10) so exp is safe), and its partial
output projection through the matching Wo column block. The host sums
the 4 partial outputs per batch and transposes (the device computes
out.T: [model_dim, seq], bf16).

All on-device layouts are transposed ([feature, seq]) so that
- projections use hsT tiles as the moving operand (N=512 matmuls),
- QK^T produces scores.T directly ([key, query]) which is what the
  AV matmul wants as its moving operand, and
- softmax normalization uses a ones-matmul partition-sum that also
  broadcasts the denominator across partitions.

Optimizations vs the 382us baseline (measured 327us):
- rope uses a sign-folded [-sin; sin] table: 3 muls + 1 add per
  projection group instead of 6 ops (DVE was the projection-chain
  serializer at 3.8us/group > the 3.4us matmul group).
- causal diagonal tiles narrow the query range to q >= (j-4ic)*128:
  fewer QK/AV matmul columns, fewer exp'd elements, and the band
  mask shrinks to a [128,128] tril on the leading columns.
- two-pass schedule: K+V projections for all strips first, then Q
  projections in REVERSE chunk order, each followed by its chunk's
  attention + Wo block. The heaviest causal chunk's long EXP chain
  starts first and hides under the remaining Q/Wo matmuls; the
  kernel ends on the lightest chunk (no ACT-bound tail hump).
  Strips for the two largest chunks stay resident; the other two
  re-load (+4.2MB DMA, fully hidden).
- warmup sized (48 matmuls) to keep the PE busy and the HAM clock
  gate at 8/8 exactly until the gating DMAs (strip0 + Wk) land.
- PSUM: pp=2, sp=3 (score tiles; lets QK run 2 ahead of EXP),
  av=1, misc=2 banks.
- output is bf16 (halves the store DMA; host sums partials in f32).
"""

import os

import numpy as np
import ml_dtypes

import concourse.bass as bass
import concourse.mybir as mybir
import concourse.tile as tile
from concourse import bacc
from concourse.bass_utils import run_bass_kernel_spmd

BF16 = mybir.dt.bfloat16
F16 = mybir.dt.float16
F32 = mybir.dt.float32
NPBF16 = ml_dtypes.bfloat16
NPF16 = np.float16

NCORES = 8
B = 2
S = 2048
HDIM = 2048
NH = 16
HD = 128
HPC = 4  # heads per core
CPB = 4  # cores per batch
SCW = 512  # s-chunk width
NSC = S // SCW  # 4
KT = HDIM // 128  # 16 k-tiles
NJT = S // 128  # 16 j-tiles
SCALE = 1.0 / np.sqrt(HD)
ROPE_BASE = 10000.0
WARM_MMS = 54

_NC_CACHE: dict[str, object] = {}
LAST_EXEC_TIME_NS = None


def _build(mode: str):
    """mode: 'causal' | 'full' | 'general'"""
    nc = bacc.Bacc("TRN2", target_bir_lowering=False, debug=False,
                   num_devices=NCORES)

    hst_d = nc.declare_dram_parameter("hst", [NSC, 128, KT * SCW], BF16, isOutput=False)
    wq_d = nc.declare_dram_parameter("wq", [128, HPC * KT * 128], BF16, isOutput=False)
    wk_d = nc.declare_dram_parameter("wk", [128, HPC * KT * 128], BF16, isOutput=False)
    wv_d = nc.declare_dram_parameter("wv", [128, KT * 512], BF16, isOutput=False)
    wo_d = nc.declare_dram_parameter("wo", [128, HPC * KT * 128], BF16, isOutput=False)
    cos_d = nc.declare_dram_parameter("cosd", [128, S], BF16, isOutput=False)
    sin_d = nc.declare_dram_parameter("sind", [128, S], BF16, isOutput=False)
    tril_d = nc.declare_dram_parameter("tril", [128, 128], F16, isOutput=False)
    if mode == "general":
        em_d = nc.declare_dram_parameter("emask", [S, S], F16, isOutput=False)
    out_d = nc.declare_dram_parameter("outT", [HDIM, S], BF16, isOutput=True)

    Exp = mybir.ActivationFunctionType.Exp

    with tile.TileContext(nc) as tc:
        with (
            tc.tile_pool(name="wpool", bufs=1) as wpool,
            tc.tile_pool(name="cpool", bufs=1) as cpool,
            tc.tile_pool(name="qkv", bufs=1) as qkvp,
            tc.tile_pool(name="strip", bufs=2) as stripp,
            tc.tile_pool(name="ropet", bufs=2) as ropet,
            tc.tile_pool(name="probs", bufs=4) as probsp,
            tc.tile_pool(name="psums", bufs=2) as psums,
            tc.tile_pool(name="recips", bufs=2) as recips,
            tc.tile_pool(name="fouts", bufs=3) as fouts,
            tc.tile_pool(name="emt", bufs=4) as emtp,
            tc.tile_pool(name="pp", bufs=2, space="PSUM") as pp_pool,
            tc.tile_pool(name="sp", bufs=3, space="PSUM") as sp_pool,
            tc.tile_pool(name="av", bufs=1, space="PSUM") as av_pool,
            tc.tile_pool(name="misc_ps", bufs=2, space="PSUM") as misc_ps,
        ):
            wq = wpool.tile([128, HPC * KT * 128], BF16, tag="wq")
            wk = wpool.tile([128, HPC * KT * 128], BF16, tag="wk")
            wv = wpool.tile([128, KT * 512], BF16, tag="wv")
            wo = wpool.tile([128, HPC * KT * 128], BF16, tag="wo")
            cos = cpool.tile([128, S], BF16, tag="cos")   # [cos; cos]
            sin = cpool.tile([128, S], BF16, tag="sin")   # [-sin; sin]
            tril = cpool.tile([128, 128], F16, tag="tril")
            ones = cpool.tile([128, 128], F16, tag="ones")
            nc.gpsimd.memset(ones[:], 1.0)

            qT = qkvp.tile([128, HPC * S], BF16, tag="qT")
            kTt = qkvp.tile([128, HPC * S], BF16, tag="kT")
            vN = qkvp.tile([128, NJT * 512], F16, tag="vN")
            oT = qkvp.tile([128, HPC * NSC * 512], BF16, tag="oT")

            # PE warmup: keep the PE busy ~3.5us+ so the HAM clock-gate
            # reaches 8/8 while the first strip + Wk DMAs land.
            warm = cpool.tile([128, 512], BF16, tag="warm")
            nc.gpsimd.memset(warm[:, 0:128], 0.0)
            wps = misc_ps.tile([128, SCW], F32, tag="mp")
            for _ in range(WARM_MMS):
                nc.tensor.matmul(wps[:], warm[:, 0:128], warm[:],
                                 start=True, stop=True)

            def proj_group(wt, dst, strip, sc, h):
                """One 16-matmul projection group + sign-folded rope.

                rope: A = pq * [cos;cos]; Bt[0:64] = pq[64:128] * (-sin);
                Bt[64:128] = pq[0:64] * (+sin); dst = A + Bt.
                (walrus only requires equal base partitions when BOTH
                inputs are SBUF, so the half-shifted reads go through the
                PSUM operand.)
                """
                pq = pp_pool.tile([128, SCW], F32, tag="pp", name="pq")
                for kt in range(KT):
                    nc.tensor.matmul(
                        pq[:],
                        wt[:, (h * KT + kt) * 128:(h * KT + kt + 1) * 128],
                        strip[:, kt * SCW:(kt + 1) * SCW],
                        start=(kt == 0), stop=(kt == KT - 1),
                    )
                cs = cos[:, sc * SCW:(sc + 1) * SCW]
                sn = sin[:, sc * SCW:(sc + 1) * SCW]
                dd = dst[:, h * S + sc * SCW: h * S + (sc + 1) * SCW]
                t1 = ropet.tile([128, SCW], BF16, tag="t1", name="t1")
                t2 = ropet.tile([128, SCW], BF16, tag="t2", name="t2")
                nc.vector.tensor_mul(t1[:], pq[:], cs)
                nc.vector.tensor_mul(t2[0:64, :], pq[64:128, :], sn[0:64, :])
                nc.vector.tensor_mul(t2[64:128, :], pq[0:64, :], sn[64:128, :])
                nc.vector.tensor_add(dd, t1[:], t2[:])

            # Pass 1: K + V projections for all strips (no attention yet).
            kept = {}
            for sc in range(NSC):
                strip = stripp.tile([128, KT * SCW], BF16, tag="st", name="strip")
                nc.sync.dma_start(strip[:], hst_d[sc])
                if sc == 0:
                    # consumption order; wq/wo are issued at the start of
                    # pass 2 so the strip DMAs are not stuck behind them
                    nc.sync.dma_start(wk[:], wk_d[:])
                    nc.sync.dma_start(cos[:], cos_d[:])
                    nc.sync.dma_start(sin[:], sin_d[:])
                    nc.sync.dma_start(wv[:], wv_d[:])
                    nc.sync.dma_start(tril[:], tril_d[:])
                kept[sc] = strip
                for h in range(HPC):
                    proj_group(wk, kTt, strip, sc, h)
                for st in range(4):
                    vp = pp_pool.tile([128, SCW], F32, tag="pp", name="vp")
                    for kt in range(KT):
                        nc.tensor.matmul(
                            vp[:],
                            strip[:, kt * SCW + st * 128: kt * SCW + (st + 1) * 128],
                            wv[:, kt * 512:(kt + 1) * 512],
                            start=(kt == 0), stop=(kt == KT - 1),
                        )
                    jt = sc * 4 + st
                    nc.scalar.copy(vN[:, jt * 512:(jt + 1) * 512], vp[:])

            def emit_wo(ic):
                for mt in range(KT):
                    fp = misc_ps.tile([128, SCW], F32, tag="mp", name="fp")
                    for h in range(HPC):
                        nc.tensor.matmul(
                            fp[:],
                            wo[:, (h * KT + mt) * 128:(h * KT + mt + 1) * 128],
                            oT[:, (h * NSC + ic) * 512:(h * NSC + ic + 1) * 512],
                            start=(h == 0), stop=(h == HPC - 1),
                        )
                    fs = fouts.tile([128, SCW], BF16, tag="fs", name="fs")
                    nc.scalar.copy(fs[:], fp[:])
                    nc.sync.dma_start(
                        out_d[mt * 128:(mt + 1) * 128, ic * SCW:(ic + 1) * SCW],
                        fs[:],
                    )

            # Pass 2: Q projection + attention + Wo, in REVERSE chunk order.
            # The heaviest causal chunk (ic = NSC-1) starts first, so its
            # long EXP chain overlaps the remaining Q projections and Wo
            # blocks; the kernel ends on the lightest chunk. Strips for the
            # two largest chunks are still resident from pass 1; the other
            # two are re-loaded (the strip pool recycles their slots).
            for ic in reversed(range(NSC)):
                if ic == NSC - 1:
                    nc.sync.dma_start(wq[:], wq_d[:])
                    nc.sync.dma_start(wo[:], wo_d[:])
                if ic >= NSC - 2:
                    strip = kept[ic]
                else:
                    strip = stripp.tile([128, KT * SCW], BF16, tag="st",
                                        name="strip")
                    nc.sync.dma_start(strip[:], hst_d[ic])
                for h in range(HPC):
                    proj_group(wq, qT, strip, ic, h)

                nj = 4 * (ic + 1) if mode == "causal" else NJT
                for h in range(HPC):
                    av = av_pool.tile([128, SCW], F32)
                    Ps = psums.tile([128, SCW], F16)
                    for j in range(nj):
                        q0 = ((j - 4 * ic) * 128
                              if mode == "causal" and j >= 4 * ic else 0)
                        sp = sp_pool.tile([128, SCW], F32)
                        nc.tensor.matmul(
                            sp[:, q0:512],
                            kTt[:, h * S + j * 128: h * S + (j + 1) * 128],
                            qT[:, h * S + ic * SCW + q0: h * S + (ic + 1) * SCW],
                            start=True, stop=True,
                        )
                        pr = probsp.tile([128, SCW], F16)
                        nc.scalar.activation(pr[:, q0:512], sp[:, q0:512],
                                             Exp, scale=float(SCALE))
                        if mode == "general":
                            emt = emtp.tile([128, SCW], F16)
                            nc.sync.dma_start(
                                emt[:],
                                em_d[j * 128:(j + 1) * 128,
                                     ic * SCW:(ic + 1) * SCW],
                            )
                            nc.vector.tensor_mul(pr[:], pr[:], emt[:])
                        elif mode == "causal" and j >= 4 * ic:
                            # within-tile causal triangle on the leading
                            # 128 query columns of the narrowed range
                            nc.vector.tensor_mul(pr[:, q0:q0 + 128],
                                                 pr[:, q0:q0 + 128],
                                                 tril[:])
                        if j == 0:
                            nc.vector.tensor_copy(Ps[:], pr[:])
                        else:
                            nc.vector.tensor_add(Ps[:, q0:512],
                                                 Ps[:, q0:512],
                                                 pr[:, q0:512])
                        nc.tensor.matmul(
                            av[:, q0:512],
                            vN[:, j * 512 + h * 128: j * 512 + (h + 1) * 128],
                            pr[:, q0:512],
                            start=(j == 0), stop=(j == nj - 1),
                        )
                    rs = misc_ps.tile([128, SCW], F32, tag="mp")
                    nc.tensor.matmul(rs[:], ones[:], Ps[:], start=True, stop=True)
                    rc = recips.tile([128, SCW], F32)
                    nc.vector.reciprocal_approx_fast(rc[:], rs[:])
                    nc.vector.tensor_mul(
                        oT[:, (h * NSC + ic) * 512:(h * NSC + ic + 1) * 512],
                        av[:], rc[:],
                    )
                emit_wo(ic)

    nc.compile()
    return nc


def _get_nc(mode: str):
    if mode not in _NC_CACHE:
        _NC_CACHE[mode] = _build(mode)
    return _NC_CACHE[mode]


def _classify_mask(m: np.ndarray) -> str:
    if not m.any():
        return "full"
    tril = np.tril(np.ones((S, S), dtype=bool))
    if np.all(m[tril] == 0.0) and np.all(m[~tril] <= -1e8):
        return "causal"
    return "general"


def kernel(hidden_states, attention_mask, position_ids, Wq, Wk, Wv, Wo):
    global LAST_EXEC_TIME_NS
    hs = np.asarray(hidden_states, dtype=np.float32)
    mask = np.asarray(attention_mask, dtype=np.float32)[0, 0]
    pos = np.asarray(position_ids)
    Wq = np.asarray(Wq, dtype=np.float32)
    Wk = np.asarray(Wk, dtype=np.float32)
    Wv = np.asarray(Wv, dtype=np.float32)
    Wo = np.asarray(Wo, dtype=np.float32)

    mode = _classify_mask(mask)
    nc = _get_nc(mode)

    # rope tables per batch: emb = concat(freqs, freqs) -> the two head-dim
    # halves share the same angle table. Sign-folded for the device:
    #   cosd = [cos; cos],  sind = [-sin; sin]   (both [128, S])
    inv_freq = 1.0 / (ROPE_BASE ** (np.arange(0, HD, 2, dtype=np.float32) / HD))
    cos_b, sin_b = [], []
    for b in range(B):
        ang = np.outer(pos[b].astype(np.float32), inv_freq)  # [S, 64]
        c = np.cos(ang).T.astype(np.float32)                 # [64, S]
        s = np.sin(ang).T.astype(np.float32)
        cos_b.append(np.concatenate([c, c], axis=0).astype(NPBF16).copy())
        sin_b.append(np.concatenate([-s, s], axis=0).astype(NPBF16).copy())

    # within-tile causal triangle: tril[p, x] = 1 if p <= x
    pidx = np.arange(128)[:, None]
    xidx = np.arange(128)[None, :]
    trilm = (pidx <= xidx).astype(NPF16)

    emask = None
    if mode == "general":
        with np.errstate(under="ignore", over="ignore"):
            emask = np.exp(mask.T.astype(np.float64)).astype(NPF16)

    in_maps = []
    for c in range(NCORES):
        b = c // CPB
        r0 = (c % CPB) * HPC * HD  # feature-row base of this core's heads

        hsb = hs[b]  # [S, HDIM]
        hst = (hsb.reshape(NSC, SCW, KT, 128).transpose(0, 3, 2, 1)
               .reshape(NSC, 128, KT * SCW).astype(NPBF16))

        Wq_s = Wq[r0:r0 + 512]  # [512, HDIM]
        wq_t = (Wq_s.reshape(HPC, 128, KT, 128).transpose(3, 0, 2, 1)
                .reshape(128, HPC * KT * 128).astype(NPBF16))
        Wk_s = Wk[r0:r0 + 512]
        wk_t = (Wk_s.reshape(HPC, 128, KT, 128).transpose(3, 0, 2, 1)
                .reshape(128, HPC * KT * 128).astype(NPBF16))
        Wv_s = Wv[r0:r0 + 512]  # [512, HDIM]
        wv_t = (Wv_s.reshape(512, KT, 128).transpose(2, 1, 0)
                .reshape(128, KT * 512).astype(NPBF16))
        Wo_s = Wo[:, r0:r0 + 512]  # [HDIM, 512]
        wo_t = (Wo_s.reshape(KT, 128, HPC, 128).transpose(3, 2, 0, 1)
                .reshape(128, HPC * KT * 128).astype(NPBF16))

        m = {
            "hst": hst, "wq": wq_t, "wk": wk_t, "wv": wv_t, "wo": wo_t,
            "cosd": cos_b[b], "sind": sin_b[b], "tril": trilm,
        }
        if mode == "general":
            m["emask"] = emask
        in_maps.append(m)

    trace = os.environ.get("BASS_KERNEL_TRACE") == "1"
    res = run_bass_kernel_spmd(nc, in_maps, core_ids=list(range(NCORES)),
                               trace=trace)
    LAST_EXEC_TIME_NS = res.exec_time_ns

    out = np.empty((B, S, HDIM), dtype=np.float32)
    for b in range(B):
        acc = res.results[CPB * b]["outT"].astype(np.float32)
        for c in range(CPB * b + 1, CPB * (b + 1)):
            acc = acc + res.results[c]["outT"].astype(np.float32)
        out[b] = acc.T
    return out


# revision 41
# speedup vs baseline: 1.1935x; 1.0046x over previous
"""Multi-head causal attention (RoPE) on 8 Trainium2 NeuronCores.

Sharding (Megatron-style): core c handles batch c//4 and the 4 heads
[4*(c%4), 4*(c%4)+4). Each core computes Q/K/V projections for its
head slice, rotary embedding, causal flash-style attention (no
max-subtraction: scores are O(10) so exp is safe), and its partial
output projection through the matching Wo column block. The host sums
the 4 partial outputs per batch and transposes (the device computes
out.T: [model_dim, seq], bf16).

All on-device layouts are transposed ([feature, seq]) so that
- projections use hsT tiles as the moving operand (N=512 matmuls),
- QK^T produces scores.T directly ([key, query]) which is what the
  AV matmul wants as its moving operand, and
- softmax normalization uses a ones-matmul partition-sum that also
  broadcasts the denominator across partitions.

Optimizations vs the 382us baseline (measured 327us):
- rope uses a sign-folded [-sin; sin] table: 3 muls + 1 add per
  projection group instead of 6 ops (DVE was the projection-chain
  serializer at 3.8us/group > the 3.4us matmul group).
- causal diagonal tiles narrow the query range to q >= (j-4ic)*128:
  fewer QK/AV matmul columns, fewer exp'd elements, and the band
  mask shrinks to a [128,128] tril on the leading columns.
- two-pass schedule: K+V projections for all strips first, then Q
  projections in REVERSE chunk order, each followed by its chunk's
  attention + Wo block. The heaviest causal chunk's long EXP chain
  starts first and hides under the remaining Q/Wo matmuls; the
  kernel ends on the lightest chunk (no ACT-bound tail hump).
  Strips for the two largest chunks stay resident; the other two
  re-load (+4.2MB DMA, fully hidden).
- warmup sized (48 matmuls) to keep the PE busy and the HAM clock
  gate at 8/8 exactly until the gating DMAs (strip0 + Wk) land.
- PSUM: pp=2, sp=3 (score tiles; lets QK run 2 ahead of EXP),
  av=1, misc=2 banks.
- output is bf16 (halves the store DMA; host sums partials in f32).
"""

import os

import numpy as np
import ml_dtypes

import concourse.bass as bass
import concourse.mybir as mybir
import concourse.tile as tile
from concourse import bacc
from concourse.bass_utils import run_bass_kernel_spmd

BF16 = mybir.dt.bfloat16
F16 = mybir.dt.float16
F32 = mybir.dt.float32
NPBF16 = ml_dtypes.bfloat16
NPF16 = np.float16

NCORES = 8
B = 2
S = 2048
HDIM = 2048
NH = 16
HD = 128
HPC = 4  # heads per core
CPB = 4  # cores per batch
SCW = 512  # s-chunk width
NSC = S // SCW  # 4
KT = HDIM // 128  # 16 k-tiles
NJT = S // 128  # 16 j-tiles
SCALE = 1.0 / np.sqrt(HD)
ROPE_BASE = 10000.0
WARM_MMS = 54

_NC_CACHE: dict[str, object] = {}
LAST_EXEC_TIME_NS = None


def _build(mode: str):
    """mode: 'causal' | 'full' | 'general'"""
    nc = bacc.Bacc("TRN2", target_bir_lowering=False, debug=False,
                   num_devices=NCORES)

    hst_d = nc.declare_dram_parameter("hst", [NSC, 128, KT * SCW], BF16, isOutput=False)
    wq_d = nc.declare_dram_parameter("wq", [128, HPC * KT * 128], BF16, isOutput=False)
    wk_d = nc.declare_dram_parameter("wk", [128, HPC * KT * 128], BF16, isOutput=False)
    wv_d = nc.declare_dram_parameter("wv", [128, KT * 512], BF16, isOutput=False)
    wo_d = nc.declare_dram_parameter("wo", [128, HPC * KT * 128], BF16, isOutput=False)
    cos_d = nc.declare_dram_parameter("cosd", [128, S], BF16, isOutput=False)
    sin_d = nc.declare_dram_parameter("sind", [128, S], BF16, isOutput=False)
    tril_d = nc.declare_dram_parameter("tril", [128, 128], F16, isOutput=False)
    if mode == "general":
        em_d = nc.declare_dram_parameter("emask", [S, S], F16, isOutput=False)
    out_d = nc.declare_dram_parameter("outT", [HDIM, S], BF16, isOutput=True)

    Exp = mybir.ActivationFunctionType.Exp

    with tile.TileContext(nc) as tc:
        with (
            tc.tile_pool(name="wpool", bufs=1) as wpool,
            tc.tile_pool(name="cpool", bufs=1) as cpool,
            tc.tile_pool(name="qkv", bufs=1) as qkvp,
            tc.tile_pool(name="strip", bufs=2) as stripp,
            tc.tile_pool(name="ropet", bufs=2) as ropet,
            tc.tile_pool(name="probs", bufs=4) as probsp,
            tc.tile_pool(name="psums", bufs=2) as psums,
            tc.tile_pool(name="recips", bufs=2) as recips,
            tc.tile_pool(name="fouts", bufs=3) as fouts,
            tc.tile_pool(name="emt", bufs=4) as emtp,
            tc.tile_pool(name="pp", bufs=2, space="PSUM") as pp_pool,
            tc.tile_pool(name="sp", bufs=3, space="PSUM") as sp_pool,
            tc.tile_pool(name="av", bufs=1, space="PSUM") as av_pool,
            tc.tile_pool(name="misc_ps", bufs=2, space="PSUM") as misc_ps,
        ):
            wq = wpool.tile([128, HPC * KT * 128], BF16, tag="wq")
            wk = wpool.tile([128, HPC * KT * 128], BF16, tag="wk")
            wv = wpool.tile([128, KT * 512], BF16, tag="wv")
            wo = wpool.tile([128, HPC * KT * 128], BF16, tag="wo")
            cos = cpool.tile([128, S], BF16, tag="cos")   # [cos; cos]
            sin = cpool.tile([128, S], BF16, tag="sin")   # [-sin; sin]
            tril = cpool.tile([128, 128], F16, tag="tril")
            ones = cpool.tile([128, 128], F16, tag="ones")

            qT = qkvp.tile([128, HPC * S], BF16, tag="qT")
            kTt = qkvp.tile([128, HPC * S], BF16, tag="kT")
            vN = qkvp.tile([128, NJT * 512], F16, tag="vN")
            oT = qkvp.tile([128, HPC * NSC * 512], BF16, tag="oT")

            # Startup-critical DMAs BEFORE the gpsimd memsets: SWDGE
            # descriptor generation shares the Q7 cores, so memsets first
            # would delay the input-DMA start that gates the kernel.
            strip0 = stripp.tile([128, KT * SCW], BF16, tag="st", name="strip")
            nc.sync.dma_start(strip0[:], hst_d[0])
            nc.sync.dma_start(wk[:], wk_d[:])
            nc.sync.dma_start(cos[:], cos_d[:])
            nc.sync.dma_start(sin[:], sin_d[:])
            nc.sync.dma_start(wv[:], wv_d[:])
            nc.sync.dma_start(tril[:], tril_d[:])

            nc.gpsimd.memset(ones[:], 1.0)
            # PE warmup: keep the PE busy ~3.5us+ so the HAM clock-gate
            # reaches 8/8 while the first strip + Wk DMAs land.
            warm = cpool.tile([128, 512], BF16, tag="warm")
            nc.gpsimd.memset(warm[:, 0:128], 0.0)
            wps = misc_ps.tile([128, SCW], F32, tag="mp")
            for _ in range(WARM_MMS):
                nc.tensor.matmul(wps[:], warm[:, 0:128], warm[:],
                                 start=True, stop=True)

            def proj_group(wt, dst, strip, sc, h):
                """One 16-matmul projection group + sign-folded rope.

                rope: A = pq * [cos;cos]; Bt[0:64] = pq[64:128] * (-sin);
                Bt[64:128] = pq[0:64] * (+sin); dst = A + Bt.
                (walrus only requires equal base partitions when BOTH
                inputs are SBUF, so the half-shifted reads go through the
                PSUM operand.)
                """
                pq = pp_pool.tile([128, SCW], F32, tag="pp", name="pq")
                for kt in range(KT):
                    nc.tensor.matmul(
                        pq[:],
                        wt[:, (h * KT + kt) * 128:(h * KT + kt + 1) * 128],
                        strip[:, kt * SCW:(kt + 1) * SCW],
                        start=(kt == 0), stop=(kt == KT - 1),
                    )
                cs = cos[:, sc * SCW:(sc + 1) * SCW]
                sn = sin[:, sc * SCW:(sc + 1) * SCW]
                dd = dst[:, h * S + sc * SCW: h * S + (sc + 1) * SCW]
                t1 = ropet.tile([128, SCW], BF16, tag="t1", name="t1")
                t2 = ropet.tile([128, SCW], BF16, tag="t2", name="t2")
                nc.vector.tensor_mul(t1[:], pq[:], cs)
                nc.vector.tensor_mul(t2[0:64, :], pq[64:128, :], sn[0:64, :])
                nc.vector.tensor_mul(t2[64:128, :], pq[0:64, :], sn[64:128, :])
                nc.vector.tensor_add(dd, t1[:], t2[:])

            # Pass 1: K + V projections for all strips (no attention yet).
            kept = {}
            for sc in range(NSC):
                if sc == 0:
                    strip = strip0
                else:
                    strip = stripp.tile([128, KT * SCW], BF16, tag="st",
                                        name="strip")
                    nc.sync.dma_start(strip[:], hst_d[sc])
                kept[sc] = strip
                for h in range(HPC):
                    proj_group(wk, kTt, strip, sc, h)
                for st in range(4):
                    vp = pp_pool.tile([128, SCW], F32, tag="pp", name="vp")
                    for kt in range(KT):
                        nc.tensor.matmul(
                            vp[:],
                            strip[:, kt * SCW + st * 128: kt * SCW + (st + 1) * 128],
                            wv[:, kt * 512:(kt + 1) * 512],
                            start=(kt == 0), stop=(kt == KT - 1),
                        )
                    jt = sc * 4 + st
                    nc.scalar.copy(vN[:, jt * 512:(jt + 1) * 512], vp[:])

            def emit_wo(ic):
                for mt in range(KT):
                    fp = misc_ps.tile([128, SCW], F32, tag="mp", name="fp")
                    for h in range(HPC):
                        nc.tensor.matmul(
                            fp[:],
                            wo[:, (h * KT + mt) * 128:(h * KT + mt + 1) * 128],
                            oT[:, (h * NSC + ic) * 512:(h * NSC + ic + 1) * 512],
                            start=(h == 0), stop=(h == HPC - 1),
                        )
                    fs = fouts.tile([128, SCW], BF16, tag="fs", name="fs")
                    nc.scalar.copy(fs[:], fp[:])
                    nc.sync.dma_start(
                        out_d[mt * 128:(mt + 1) * 128, ic * SCW:(ic + 1) * SCW],
                        fs[:],
                    )

            # Pass 2: Q projection + attention + Wo, in REVERSE chunk order.
            # The heaviest causal chunk (ic = NSC-1) starts first, so its
            # long EXP chain overlaps the remaining Q projections and Wo
            # blocks; the kernel ends on the lightest chunk. Strips for the
            # two largest chunks are still resident from pass 1; the other
            # two are re-loaded (the strip pool recycles their slots).
            for ic in reversed(range(NSC)):
                if ic == NSC - 1:
                    nc.sync.dma_start(wq[:], wq_d[:])
                    nc.sync.dma_start(wo[:], wo_d[:])
                if ic >= NSC - 2:
                    strip = kept[ic]
                else:
                    strip = stripp.tile([128, KT * SCW], BF16, tag="st",
                                        name="strip")
                    nc.sync.dma_start(strip[:], hst_d[ic])
                for h in range(HPC):
                    proj_group(wq, qT, strip, ic, h)

                nj = 4 * (ic + 1) if mode == "causal" else NJT
                for h in range(HPC):
                    av = av_pool.tile([128, SCW], F32)
                    Ps = psums.tile([128, SCW], F16)
                    for j in range(nj):
                        q0 = ((j - 4 * ic) * 128
                              if mode == "causal" and j >= 4 * ic else 0)
                        sp = sp_pool.tile([128, SCW], F32)
                        nc.tensor.matmul(
                            sp[:, q0:512],
                            kTt[:, h * S + j * 128: h * S + (j + 1) * 128],
                            qT[:, h * S + ic * SCW + q0: h * S + (ic + 1) * SCW],
                            start=True, stop=True,
                        )
                        pr = probsp.tile([128, SCW], F16)
                        nc.scalar.activation(pr[:, q0:512], sp[:, q0:512],
                                             Exp, scale=float(SCALE))
                        if mode == "general":
                            emt = emtp.tile([128, SCW], F16)
                            nc.sync.dma_start(
                                emt[:],
                                em_d[j * 128:(j + 1) * 128,
                                     ic * SCW:(ic + 1) * SCW],
                            )
                            nc.vector.tensor_mul(pr[:], pr[:], emt[:])
                        elif mode == "causal" and j >= 4 * ic:
                            # within-tile causal triangle on the leading
                            # 128 query columns of the narrowed range
                            nc.vector.tensor_mul(pr[:, q0:q0 + 128],
                                                 pr[:, q0:q0 + 128],
                                                 tril[:])
                        if j == 0:
                            nc.vector.tensor_copy(Ps[:], pr[:])
                        else:
                            nc.vector.tensor_add(Ps[:, q0:512],
                                                 Ps[:, q0:512],
                                                 pr[:, q0:512])
                        nc.tensor.matmul(
                            av[:, q0:512],
                            vN[:, j * 512 + h * 128: j * 512 + (h + 1) * 128],
                            pr[:, q0:512],
                            start=(j == 0), stop=(j == nj - 1),
                        )
                    rs = misc_ps.tile([128, SCW], F32, tag="mp")
                    nc.tensor.matmul(rs[:], ones[:], Ps[:], start=True, stop=True)
                    rc = recips.tile([128, SCW], F32)
                    nc.vector.reciprocal_approx_fast(rc[:], rs[:])
                    nc.vector.tensor_mul(
                        oT[:, (h * NSC + ic) * 512:(h * NSC + ic + 1) * 512],
                        av[:], rc[:],
                    )
                emit_wo(ic)

    nc.compile()
    return nc


def _get_nc(mode: str):
    if mode not in _NC_CACHE:
        _NC_CACHE[mode] = _build(mode)
    return _NC_CACHE[mode]


def _classify_mask(m: np.ndarray) -> str:
    if not m.any():
        return "full"
    tril = np.tril(np.ones((S, S), dtype=bool))
    if np.all(m[tril] == 0.0) and np.all(m[~tril] <= -1e8):
        return "causal"
    return "general"


def kernel(hidden_states, attention_mask, position_ids, Wq, Wk, Wv, Wo):
    global LAST_EXEC_TIME_NS
    hs = np.asarray(hidden_states, dtype=np.float32)
    mask = np.asarray(attention_mask, dtype=np.float32)[0, 0]
    pos = np.asarray(position_ids)
    Wq = np.asarray(Wq, dtype=np.float32)
    Wk = np.asarray(Wk, dtype=np.float32)
    Wv = np.asarray(Wv, dtype=np.float32)
    Wo = np.asarray(Wo, dtype=np.float32)

    mode = _classify_mask(mask)
    nc = _get_nc(mode)

    # rope tables per batch: emb = concat(freqs, freqs) -> the two head-dim
    # halves share the same angle table. Sign-folded for the device:
    #   cosd = [cos; cos],  sind = [-sin; sin]   (both [128, S])
    inv_freq = 1.0 / (ROPE_BASE ** (np.arange(0, HD, 2, dtype=np.float32) / HD))
    cos_b, sin_b = [], []
    for b in range(B):
        ang = np.outer(pos[b].astype(np.float32), inv_freq)  # [S, 64]
        c = np.cos(ang).T.astype(np.float32)                 # [64, S]
        s = np.sin(ang).T.astype(np.float32)
        cos_b.append(np.concatenate([c, c], axis=0).astype(NPBF16).copy())
        sin_b.append(np.concatenate([-s, s], axis=0).astype(NPBF16).copy())

    # within-tile causal triangle: tril[p, x] = 1 if p <= x
    pidx = np.arange(128)[:, None]
    xidx = np.arange(128)[None, :]
    trilm = (pidx <= xidx).astype(NPF16)

    emask = None
    if mode == "general":
        with np.errstate(under="ignore", over="ignore"):
            emask = np.exp(mask.T.astype(np.float64)).astype(NPF16)

    in_maps = []
    for c in range(NCORES):
        b = c // CPB
        r0 = (c % CPB) * HPC * HD  # feature-row base of this core's heads

        hsb = hs[b]  # [S, HDIM]
        hst = (hsb.reshape(NSC, SCW, KT, 128).transpose(0, 3, 2, 1)
               .reshape(NSC, 128, KT * SCW).astype(NPBF16))

        Wq_s = Wq[r0:r0 + 512]  # [512, HDIM]
        wq_t = (Wq_s.reshape(HPC, 128, KT, 128).transpose(3, 0, 2, 1)
                .reshape(128, HPC * KT * 128).astype(NPBF16))
        Wk_s = Wk[r0:r0 + 512]
        wk_t = (Wk_s.reshape(HPC, 128, KT, 128).transpose(3, 0, 2, 1)
                .reshape(128, HPC * KT * 128).astype(NPBF16))
        Wv_s = Wv[r0:r0 + 512]  # [512, HDIM]
        wv_t = (Wv_s.reshape(512, KT, 128).transpose(2, 1, 0)
                .reshape(128, KT * 512).astype(NPBF16))
        Wo_s = Wo[:, r0:r0 + 512]  # [HDIM, 512]
        wo_t = (Wo_s.reshape(KT, 128, HPC, 128).transpose(3, 2, 0, 1)
                .reshape(128, HPC * KT * 128).astype(NPBF16))

        m = {
            "hst": hst, "wq": wq_t, "wk": wk_t, "wv": wv_t, "wo": wo_t,
            "cosd": cos_b[b], "sind": sin_b[b], "tril": trilm,
        }
        if mode == "general":
            m["emask"] = emask
        in_maps.append(m)

    trace = os.environ.get("BASS_KERNEL_TRACE") == "1"
    res = run_bass_kernel_spmd(nc, in_maps, core_ids=list(range(NCORES)),
                               trace=trace)
    LAST_EXEC_TIME_NS = res.exec_time_ns

    out = np.empty((B, S, HDIM), dtype=np.float32)
    for b in range(B):
        acc = res.results[CPB * b]["outT"].astype(np.float32)
        for c in range(CPB * b + 1, CPB * (b + 1)):
            acc = acc + res.results[c]["outT"].astype(np.float32)
        out[b] = acc.T
    return out
